# revision 4
# baseline (speedup 1.0000x reference)
"""Trainium2 Bass kernel for nn_NeibRoutLayer (capsule-routing GNN message passing).

Strategy (8 NeuronCores, SPMD, no collectives, no device-side gathers):
  - Nodes padded to 50176 = 8 cores x 49 tiles x 128. Each core owns a
    contiguous 6272-node range; edges are assigned to the core/tile of their
    TARGET (host-side argsort), so the segment-sum is fully core/tile-local.
  - All iteration-invariant per-edge data is prebuilt on the host and
    streamed from DRAM as ONE contiguous bf16 tensor per tile:
      z_t  [128e, cf*128f]  rows z = l2norm(x)[src] in edge-major chunk layout
      A_t  [128n, cf*128e]  one-hot gather matrices  (A[n,e] = trg_e == n)
      S_t  [128e, cf*128n]  one-hot scatter matrices (S[e,n] = trg_e == n)
  - u lives in SBUF for the whole kernel (bf16 [128, 6272] per core).
    Per routing iteration, per node tile (all engines pipelined):
      ug   = A_ch^T @ u_tile          per chunk      (PE matmul, bf16->f32 PSUM)
      tm   = z * ug                                  (DVE, bf16 out)
      p    = reduce_16(tm)                           (DVE)
      w    = exp(p)                                  (ACT -- Exp only)
      rinv = 1 / reduce_8(w)                         (DVE)
      wn   = w * rinv                                (DVE, bf16)
      msg  = z * broadcast_16(wn)                    (GPSIMD, bf16)
      acc  = sum_ch S_ch^T @ msg_ch                  (PE matmul, f32 PSUM)
      uraw = acc + xc                                (DVE)
      n2   = reduce_16(uraw^2)                       (GPSIMD square + DVE reduce)
    then a short phase B per tile: u = uraw / sqrt(n2)  (ACT Sqrt + DVE),
    keeping the ACT engine on a single activation-table set per phase.

kernel(**inputs) takes the FULL inputs and returns the FULL output.
"""

import sys
from contextlib import ExitStack

sys.path.insert(0, "/opt/trn_rl_repo")

import numpy as np
import ml_dtypes

import concourse.bacc as bacc
import concourse.bass as bass
import concourse.tile as tile
from concourse import mybir
from concourse.bass_utils import run_bass_kernel_spmd

# ---------------------------------------------------------------- constants
N_NODES = 50000
D = 128          # feature dim
C = 8            # capsules
DPC = 16         # dims per capsule
NITER = 3
NCORES = 8
T_TILES = 49     # node tiles per core
OWN = T_TILES * 128
NPAD = NCORES * OWN

F32 = mybir.dt.float32
BF16 = mybir.dt.bfloat16
AF = mybir.ActivationFunctionType
ALU = mybir.AluOpType
BF = ml_dtypes.bfloat16


# ---------------------------------------------------------------- CPU prep
def _prepare(x, edge_index):
    """Host-side (untimed) preprocessing: sort edges by target, build per-core
    bf16 streams [z | A | S] per tile plus the xc table."""
    src = np.asarray(edge_index[0], dtype=np.int64)
    trg = np.asarray(edge_index[1], dtype=np.int64)

    order = np.argsort(trg, kind="stable")
    trg_s = trg[order]
    src_s = src[order]

    n_gtiles = NPAD // 128
    bounds = np.searchsorted(trg_s, np.arange(n_gtiles + 1) * 128)
    tile_cnt = bounds[1:] - bounds[:-1]
    cf = int(np.ceil(max(tile_cnt.max(), 1) / 128))  # chunks per tile
    spt = cf * 128                                   # padded slots per tile

    x_pad = np.ones((NPAD, D), dtype=np.float32)
    x_pad[:N_NODES] = np.asarray(x, dtype=np.float32)

    # xc = per-capsule l2norm (matches torch fn.normalize eps semantics)
    v = x_pad.reshape(NPAD, C, DPC)
    n = np.linalg.norm(v, axis=-1, keepdims=True)
    xc = (v / np.maximum(n, 1e-12)).reshape(NPAD, D).astype(np.float32)

    z_all = xc[src_s]                                # [E, D] f32

    in_maps = []
    for c in range(NCORES):
        stream = np.zeros((128, T_TILES * 3 * spt), dtype=BF)
        for j in range(T_TILES):
            g = c * T_TILES + j
            s, e = bounds[g], bounds[g + 1]
            cnt = e - s
            base = j * 3 * spt

            # z tile: [cf,128e,D] -> [128e, cf*D]
            zt = np.zeros((cf * 128, D), dtype=np.float32)
            zt[:cnt] = z_all[s:e]
            stream[:, base:base + spt] = (
                zt.reshape(cf, 128, D).transpose(1, 0, 2).reshape(128, spt)
                .astype(BF))

            # one-hot M[k, n] = (trg_local[k] == n)
            M = np.zeros((cf * 128, 128), dtype=np.float32)
            tl = (trg_s[s:e] - g * 128).astype(np.int64)
            M[np.arange(cnt), tl] = 1.0
            M3 = M.reshape(cf, 128, 128)
            # A: [n, cf*e]
            stream[:, base + spt:base + 2 * spt] = (
                M3.transpose(2, 0, 1).reshape(128, spt).astype(BF))
            # S: [e, cf*n]
            stream[:, base + 2 * spt:base + 3 * spt] = (
                M3.transpose(1, 0, 2).reshape(128, spt).astype(BF))

        xc_own = xc[c * OWN:(c + 1) * OWN]
        xc_pm = (xc_own.reshape(T_TILES, 128, D).transpose(1, 0, 2)
                 .reshape(128, T_TILES * D))

        in_maps.append({
            "stream": stream,
            "xcbf": xc_pm.astype(BF),
        })
    return cf, in_maps


# ---------------------------------------------------------------- device code
def _build(cf, niter=NITER):
    """Build the SPMD Bass program (identical on all 8 cores)."""
    spt = cf * 128

    nc = bacc.Bacc("TRN2", target_bir_lowering=False, debug=False,
                   num_devices=NCORES)

    stream_in = nc.dram_tensor("stream", [128, T_TILES * 3 * spt], BF16,
                               kind="ExternalInput").ap()
    xcbf_in = nc.dram_tensor("xcbf", [128, T_TILES * D], BF16,
                             kind="ExternalInput").ap()
    u_out = nc.dram_tensor("u_out", [128, T_TILES * D], F32,
                           kind="ExternalOutput").ap()

    with tile.TileContext(nc) as tc, ExitStack() as ctx:
        persist = ctx.enter_context(tc.tile_pool(name="persist", bufs=1))
        xc_sb = persist.tile([128, T_TILES * 128], BF16, tag="xc")
        ubf_sb = persist.tile([128, T_TILES * 128], BF16, tag="ubf")
        uraw_sb = persist.tile([128, T_TILES * 128], F32, tag="uraw")
        n2_sb = persist.tile([128, T_TILES * C], F32, tag="n2")

        nc.sync.dma_start(out=xc_sb, in_=xcbf_in[:])
        nc.sync.dma_start(out=ubf_sb, in_=xcbf_in[:])

        stream = ctx.enter_context(tc.tile_pool(name="stream", bufs=3))
        work = ctx.enter_context(tc.tile_pool(name="work", bufs=2))
        small = ctx.enter_context(tc.tile_pool(name="small", bufs=3))
        psum_tp = ctx.enter_context(
            tc.tile_pool(name="psum", bufs=2, space="PSUM"))

        SEG = 4  # chunks per PSUM segment ([128, 512] f32 = one PSUM bank)

        for it in range(niter):
            last = it == niter - 1
            for t in range(T_TILES):
                st = stream.tile([128, 3 * spt], BF16, tag="st")
                nc.sync.dma_start(
                    out=st, in_=stream_in[:, t * 3 * spt:(t + 1) * 3 * spt])
                z_ap = st[:, 0:spt]
                a_ap = st[:, spt:2 * spt]
                s_ap = st[:, 2 * spt:3 * spt]
                ut = ubf_sb[:, bass.ts(t, 128)]

                tm = work.tile([128, spt], BF16, tag="tm")
                c0 = 0
                while c0 < cf:
                    nch = min(SEG, cf - c0)
                    ug = psum_tp.tile([128, nch * 128], F32, tag="ug")
                    for ch in range(nch):
                        nc.tensor.matmul(
                            out=ug[:, bass.ts(ch, 128)],
                            lhsT=a_ap[:, bass.ts(c0 + ch, 128)],
                            rhs=ut, start=True, stop=True)
                    nc.vector.tensor_tensor(
                        out=tm[:, c0 * 128:(c0 + nch) * 128],
                        in0=z_ap[:, c0 * 128:(c0 + nch) * 128],
                        in1=ug, op=ALU.mult)
                    c0 += nch

                pav = small.tile([128, cf * C], F32, tag="pav")
                nc.vector.reduce_sum(
                    out=pav, in_=tm.rearrange("p (a b) -> p a b", b=DPC),
                    axis=mybir.AxisListType.X)
                wexp = small.tile([128, cf * C], F32, tag="wexp")
                nc.scalar.activation(wexp, pav, AF.Exp)
                s8 = small.tile([128, cf], F32, tag="s8")
                nc.vector.reduce_sum(
                    out=s8, in_=wexp.rearrange("p (a b) -> p a b", b=C),
                    axis=mybir.AxisListType.X)
                rinv = small.tile([128, cf], F32, tag="rinv")
                nc.vector.reciprocal(rinv, s8)
                wn = small.tile([128, cf * C], BF16, tag="wn")
                nc.vector.tensor_tensor(
                    out=wn.rearrange("p (a b) -> p a b", b=C),
                    in0=wexp.rearrange("p (a b) -> p a b", b=C),
                    in1=rinv.to_broadcast([128, cf, C]),
                    op=ALU.mult)
                msg = work.tile([128, spt], BF16, tag="msg")
                nc.gpsimd.tensor_tensor(
                    out=msg.rearrange("p (a b) -> p a b", b=DPC),
                    in0=z_ap.rearrange("p (a b) -> p a b", b=DPC),
                    in1=wn.to_broadcast([128, cf * C, DPC]),
                    op=ALU.mult)

                acc = psum_tp.tile([128, 128], F32, tag="acc")
                for ch in range(cf):
                    nc.tensor.matmul(out=acc,
                                     lhsT=s_ap[:, bass.ts(ch, 128)],
                                     rhs=msg[:, bass.ts(ch, 128)],
                                     start=(ch == 0), stop=(ch == cf - 1))
                # uraw = acc + xc
                nc.vector.scalar_tensor_tensor(
                    out=uraw_sb[:, bass.ts(t, 128)],
                    in0=acc, scalar=1.0, in1=xc_sb[:, bass.ts(t, 128)],
                    op0=ALU.mult, op1=ALU.add)
                sq = work.tile([128, 128], F32, tag="sq")
                nc.gpsimd.tensor_tensor(
                    out=sq, in0=uraw_sb[:, bass.ts(t, 128)],
                    in1=uraw_sb[:, bass.ts(t, 128)], op=ALU.mult)
                nc.vector.reduce_sum(
                    out=n2_sb[:, bass.ts(t, C)],
                    in_=sq.rearrange("p (a b) -> p a b", b=DPC),
                    axis=mybir.AxisListType.X)

            # ---- phase B: u = uraw / sqrt(n2)  (ACT switches to Sqrt table)
            for t in range(T_TILES):
                nrm = small.tile([128, C], F32, tag="nrm")
                nc.scalar.activation(nrm, n2_sb[:, bass.ts(t, C)], AF.Sqrt)
                rn = small.tile([128, C], F32, tag="rn")
                nc.vector.reciprocal(rn, nrm)
                if last:
                    uo = work.tile([128, 128], F32, tag="uo")
                    nc.vector.tensor_tensor(
                        out=uo.rearrange("p (a b) -> p a b", b=DPC),
                        in0=uraw_sb[:, bass.ts(t, 128)].rearrange(
                            "p (a b) -> p a b", b=DPC),
                        in1=rn.to_broadcast([128, C, DPC]),
                        op=ALU.mult)
                    nc.sync.dma_start(out=u_out[:, bass.ts(t, 128)], in_=uo)
                else:
                    nc.vector.tensor_tensor(
                        out=ubf_sb[:, bass.ts(t, 128)].rearrange(
                            "p (a b) -> p a b", b=DPC),
                        in0=uraw_sb[:, bass.ts(t, 128)].rearrange(
                            "p (a b) -> p a b", b=DPC),
                        in1=rn.to_broadcast([128, C, DPC]),
                        op=ALU.mult)

    nc.compile()
    return nc


_CACHE = {}


def _get_program(cf, niter=NITER):
    if (cf, niter) not in _CACHE:
        _CACHE[(cf, niter)] = _build(cf, niter)
    return _CACHE[(cf, niter)]


def _run(nc, in_maps):
    return run_bass_kernel_spmd(nc, in_maps, list(range(NCORES)))


def kernel(**inputs):
    x = inputs["x"]
    edge_index = inputs["edge_index"]
    cf, in_maps = _prepare(x, edge_index)
    nc = _get_program(cf)
    res = _run(nc, in_maps)
    outs = []
    for c in range(NCORES):
        o = res.results[c]["u_out"]              # [128, T*128] partition-major
        outs.append(np.transpose(o.reshape(128, T_TILES, D), (1, 0, 2))
                    .reshape(OWN, D))
    out = np.concatenate(outs, axis=0)
    return np.ascontiguousarray(out[:N_NODES]).astype(np.float32)
